# revision 1
# baseline (speedup 1.0000x reference)
"""TreeLSTM (complete binary tree, S=255, B=64) on 8 trn2 NeuronCores.

Sharding: data-parallel over batch (8 examples per core); every core holds the
full embedding table and weights. No collectives needed (trees independent).

Per-core pipeline (all feature dims chunked by 100 partitions):
  1. dma_gather embedding rows (host-padded to 256 f32) by token id.
  2. PE-transpose gathered rows -> emb^T [feat, node*batch] bf16.
  3. wx_iou = W_iou^T emb^T (+b) via bf16 matmuls; leaf gates consume the
     PSUM directly (fused); internal wx materialized in SBUF.
  4. 8 tree levels, deepest first: child-sum via strided even/odd adds,
     forget gates, iou gates, cell/hidden updates. tanh computed as
     2*sigmoid(2x)-1 to stay on the sigmoid ACT table.
  5. Head: logits = W_out^T h_root + b_out, log_softmax via exp + poly-log.
"""

import os
import sys
import types

import numpy as np


def _install_axon_hook():
    """Register the NTFF profile hook so BASS_TRACE=1 tracing works."""
    try:
        import antenv

        if "antenv.axon_hooks" in sys.modules:
            return
        hooks = types.ModuleType("antenv.axon_hooks")
        hooks._hook = None

        def set_axon_ntff_profile_hook(h):
            hooks._hook = h

        def get_axon_ntff_profile_hook():
            return hooks._hook

        hooks.set_axon_ntff_profile_hook = set_axon_ntff_profile_hook
        hooks.get_axon_ntff_profile_hook = get_axon_ntff_profile_hook
        sys.modules["antenv.axon_hooks"] = hooks
        antenv.axon_hooks = hooks
        try:
            from trn_agent_boot.trn_boot import _ntff_profile_via_ctypes

            set_axon_ntff_profile_hook(
                _ntff_profile_via_ctypes("/opt/axon/libaxon_pjrt.so")
            )
        except Exception:
            pass
    except Exception:
        pass


_install_axon_hook()

from contextlib import ExitStack  # noqa: E402

import concourse.bacc as bacc  # noqa: E402
import concourse.mybir as mybir  # noqa: E402
import concourse.tile as tile  # noqa: E402
from concourse import library_config  # noqa: E402
from concourse.bass_utils import run_bass_kernel_spmd  # noqa: E402

F32 = mybir.dt.float32
BF16 = mybir.dt.bfloat16
I16 = mybir.dt.int16
AF = mybir.ActivationFunctionType
OP = mybir.AluOpType

E, HID, NCLS = 200, 300, 2
B, S, V = 64, 255, 32000
NCORES, BL = 8, 8
CH = 100  # feature chunk (partitions)
KE, KH, M9 = 2, 3, 9  # K-chunks for E(200), H(300); M-chunks for 3H(900)
ROWPAD = 256  # padded embedding row, f32 elems
NLEAF = 1024  # 128 leaf nodes * 8 batch
NINT = 1016  # 127 internal nodes * 8
NINTP = 1024  # padded

TAPS = bool(int(os.environ.get("BASS_KERNEL_TAPS", "0")))

# ln(1+u) on u in [0,1]: minimax-ish poly (deg 5, Chebyshev fit, err ~5e-5)
_LOG_COEF = None


def _log_coeffs():
    global _LOG_COEF
    if _LOG_COEF is None:
        u = np.linspace(0.0, 1.0, 20001)
        # fit ln(1+u)/u = c0 + c1 u + ... + c4 u^4
        y = np.log1p(u[1:]) / u[1:]
        cs = np.polynomial.chebyshev.Chebyshev.fit(u[1:], y, 4).convert().coef
        _LOG_COEF = list(cs)  # c0..c4
    return _LOG_COEF


def _fl(ap):
    """Flatten all free dims of a contiguous AP (DVE/ACT fast path)."""
    nd = len(ap.shape)
    if nd <= 2:
        return ap
    pat = {
        3: "p a b -> p (a b)",
        4: "p a b c -> p (a b c)",
        5: "p a b c d -> p (a b c d)",
    }[nd]
    return ap.rearrange(pat)


def _build():
    nc = bacc.Bacc(
        "TRN2", target_bir_lowering=False, debug=False, num_devices=NCORES,
        enable_asserts=False,
    )
    embp = nc.dram_tensor("embp", [V, ROWPAD], F32, kind="ExternalInput")
    idxl = nc.dram_tensor("idxl", [128, 64], I16, kind="ExternalInput")
    idxi = nc.dram_tensor("idxi", [128, 64], I16, kind="ExternalInput")
    wiou_d = nc.dram_tensor("wiou", [E, 3 * HID], F32, kind="ExternalInput")
    uiou_d = nc.dram_tensor("uiou", [HID, 3 * HID], F32, kind="ExternalInput")
    biou_d = nc.dram_tensor("biou", [CH, M9], F32, kind="ExternalInput")
    wf_d = nc.dram_tensor("wf", [E, HID], F32, kind="ExternalInput")
    uf_d = nc.dram_tensor("uf", [HID, HID], F32, kind="ExternalInput")
    bf_d = nc.dram_tensor("bf", [CH, KH], F32, kind="ExternalInput")
    wout_d = nc.dram_tensor("wout", [HID, NCLS], F32, kind="ExternalInput")
    bout_d = nc.dram_tensor("bout", [1, NCLS], F32, kind="ExternalInput")
    ident_d = nc.dram_tensor("ident", [128, 128], F32, kind="ExternalInput")
    out_d = nc.dram_tensor("out", [BL, NCLS], F32, kind="ExternalOutput")

    taps = {}
    if TAPS:
        taps["embl"] = nc.dram_tensor("tap_embl", [CH, KE, NLEAF], BF16, kind="ExternalOutput")
        taps["embi"] = nc.dram_tensor("tap_embi", [CH, KE, NINTP], BF16, kind="ExternalOutput")
        taps["wxf"] = nc.dram_tensor("tap_wxf", [CH, KH, NINTP], BF16, kind="ExternalOutput")
        taps["H7"] = nc.dram_tensor("tap_H7", [CH, KH, NLEAF], BF16, kind="ExternalOutput")
        taps["C7"] = nc.dram_tensor("tap_C7", [CH, KH, NLEAF], BF16, kind="ExternalOutput")
        taps["H5"] = nc.dram_tensor("tap_H5", [CH, KH, 256], BF16, kind="ExternalOutput")
        taps["H0"] = nc.dram_tensor("tap_H0", [CH, KH, 8], BF16, kind="ExternalOutput")

    with tile.TileContext(nc) as tc, ExitStack() as ctx:
        const = ctx.enter_context(tc.tile_pool(name="const", bufs=1))
        stage = ctx.enter_context(tc.tile_pool(name="stage", bufs=2))
        gat = ctx.enter_context(tc.tile_pool(name="gat", bufs=1))
        acts = ctx.enter_context(tc.tile_pool(name="acts", bufs=1))
        tr = ctx.enter_context(tc.tile_pool(name="tr", bufs=1))
        sm = ctx.enter_context(tc.tile_pool(name="sm", bufs=1))

        nc.gpsimd.load_library(library_config.mlp)

        # ---- index loads + early sigmoid table load ----
        idxl_t = const.tile([128, 64], I16)
        nc.sync.dma_start(idxl_t[:], idxl[:])
        idxi_t = const.tile([128, 64], I16)
        nc.sync.dma_start(idxi_t[:], idxi[:])
        ones_t = const.tile([1, BL], BF16)
        nc.vector.memset(ones_t[:], 1.0)
        dummy_t = const.tile([1, BL], BF16)
        nc.scalar.activation(dummy_t[:], ones_t[:], AF.Sigmoid)

        # ---- gathers, leaf in 256-token chunks first ----
        g_l = gat.tile([128, 8, ROWPAD], F32, tag="g_l")
        g_i = gat.tile([128, 8, ROWPAD], F32, tag="g_i")
        for h in range(4):
            nc.gpsimd.dma_gather(
                g_l[:, 2 * h : 2 * h + 2, :], embp[:],
                idxl_t[:, 16 * h : 16 * h + 16], 256, 256, ROWPAD,
            )
        for h in range(2):
            nc.gpsimd.dma_gather(
                g_i[:, 4 * h : 4 * h + 4, :], embp[:],
                idxi_t[:, 32 * h : 32 * h + 32], 512, 512, ROWPAD,
            )

        # ---- remaining consts ----
        ident_t = const.tile([128, 128], F32)
        nc.sync.dma_start(ident_t[:], ident_d[:])
        biou_t = const.tile([CH, M9], F32)
        nc.sync.dma_start(biou_t[:], biou_d[:])
        bf_t = const.tile([CH, KH], F32)
        nc.sync.dma_start(bf_t[:], bf_d[:])
        # u-gate needs sigma(2x+2b): pre-doubled bias
        biou2_t = const.tile([CH, 3], F32)
        nc.vector.tensor_scalar_mul(biou2_t[:], biou_t[:, 6:9], 2.0)

        bout_s = stage.tile([1, NCLS], F32, tag="bout_s")
        nc.sync.dma_start(bout_s[:], bout_d[:])
        bout_t = const.tile([1, NCLS], BF16)
        nc.vector.tensor_copy(bout_t[:], bout_s[:])

        # ---- weights: DMA f32 + cast bf16 on ACT (idle early) ----
        def load_w(dram, rows, cols, kchunks, name):
            t = const.tile([CH, kchunks, cols], BF16, tag=name, name=name)
            for k in range(kchunks):
                st = stage.tile([CH, cols], F32, tag="wstage", name="wst")
                nc.sync.dma_start(st[:], dram[CH * k : CH * (k + 1), :])
                nc.vector.tensor_copy(t[:, k, :], st[:])
            return t

        wiou_t = load_w(wiou_d, E, 3 * HID, KE, "wiou")
        wf_t = load_w(wf_d, E, HID, KE, "wf")
        uiou_t = load_w(uiou_d, HID, 3 * HID, KH, "uiou")
        uf_t = load_w(uf_d, HID, HID, KH, "uf")
        wout_t = load_w(wout_d, HID, NCLS, KH, "wout")

        # leaf gate tiles [CH, 3(mchunk), 128(node), 8(batch)]
        gi7 = acts.tile([CH, 3, 128, 8], BF16, tag="gi7")
        go7 = acts.tile([CH, 3, 128, 8], BF16, tag="go7")
        gu7 = acts.tile([CH, 3, 128, 8], BF16, tag="gu7")
        leaf_g = (gi7, go7, gu7)
        wxf_t = acts.tile([CH, KH, NINTP], BF16, tag="wxf")

        with ExitStack() as ps1:
            ps_tp = ps1.enter_context(
                tc.tile_pool(name="ps_tp", bufs=4, space="PSUM")
            )
            ps_wx = ps1.enter_context(
                tc.tile_pool(name="ps_wx", bufs=3, space="PSUM")
            )

            # ---- transposes: bank-aligned psum per transpose ----
            def transpose_half(g_t, name):
                embt = acts.tile([CH, KE, 1024], BF16, tag=name, name=name)
                for s in range(8):
                    for k in range(KE):
                        pt = ps_tp.tile([CH, 128], F32, tag="pt", name="pt")
                        nc.tensor.transpose(
                            pt[:],
                            g_t[:, s, CH * k : CH * (k + 1)],
                            ident_t[:],
                        )
                        nc.vector.tensor_copy(
                            embt[:, k, 128 * s : 128 * (s + 1)], pt[:]
                        )
                return embt

            embl_t = transpose_half(g_l, "embl")
            embi_t = transpose_half(g_i, "embi")

            # ---- leaf gates fused with wx matmuls ----
            for nt in range(2):
                for g in (0, 2, 1):
                    for m in range(3):
                        m9 = 3 * g + m
                        ps = ps_wx.tile([CH, 64, 8], F32, tag="ps_wx", name="psw")
                        for k in range(KE):
                            nc.tensor.matmul(
                                ps[:],
                                wiou_t[:, k, CH * m9 : CH * (m9 + 1)],
                                embl_t[:, k, 512 * nt : 512 * (nt + 1)],
                                start=(k == 0),
                                stop=(k == KE - 1),
                            )
                        if g == 2:
                            bias, scale = biou2_t[:, m : m + 1], 2.0
                        else:
                            bias, scale = biou_t[:, m9 : m9 + 1], 1.0
                        nc.scalar.activation(
                            leaf_g[g][:, m, 64 * nt : 64 * (nt + 1), :],
                            ps[:],
                            AF.Sigmoid,
                            bias=bias,
                            scale=scale,
                        )

            # ---- internal wx: wxf first (level 6 needs it first) ----
            for m in range(KH):
                for nt in range(2):
                    ps = ps_wx.tile([CH, 512], F32, tag="ps_wx", name="psw")
                    for k in range(KE):
                        nc.tensor.matmul(
                            ps[:],
                            wf_t[:, k, CH * m : CH * (m + 1)],
                            embi_t[:, k, 512 * nt : 512 * (nt + 1)],
                            start=(k == 0),
                            stop=(k == KE - 1),
                        )
                    nc.scalar.activation(
                        wxf_t[:, m, 512 * nt : 512 * (nt + 1)],
                        ps[:],
                        AF.Identity,
                        bias=bf_t[:, m : m + 1],
                    )
        # ---- leaf cell/hidden (no psum needed) ----
        H = {}
        C = {}
        C[7] = acts.tile([CH, KH, 128, 8], BF16, tag="C7", name="C7")
        H[7] = acts.tile([CH, KH, 128, 8], BF16, tag="H7", name="H7")
        for h in range(2):
            nsl = slice(64 * h, 64 * (h + 1))
            gih, guh, goh = gi7[:, :, nsl, :], gu7[:, :, nsl, :], go7[:, :, nsl, :]
            tci7 = tr.tile([CH, KH, 64, 8], BF16, tag="tci", name="tci7")
            nc.vector.tensor_mul(tci7[:], gih, guh)
            # c = i*u = i*(2*sigma(2x)-1) = 2*(i*su) - i
            nc.vector.scalar_tensor_tensor(
                C[7][:, :, nsl, :], tci7[:], 2.0, gih, OP.mult, OP.subtract
            )
            sc7 = tr.tile([CH, KH, 64, 8], BF16, tag="sc", name="sc7")
            nc.scalar.activation(
                sc7[:], C[7][:, :, nsl, :], AF.Sigmoid, scale=2.0
            )
            th7 = tr.tile([CH, KH, 64, 8], BF16, tag="th", name="th7")
            nc.vector.tensor_mul(th7[:], goh, sc7[:])
            nc.vector.scalar_tensor_tensor(
                H[7][:, :, nsl, :], th7[:], 2.0, goh, OP.mult, OP.subtract
            )

        # ---- tree levels 6..0 ----
        with ExitStack() as ps2:
            ps_uf = ps2.enter_context(
                tc.tile_pool(name="ps_uf", bufs=2, space="PSUM")
            )
            ps_io = ps2.enter_context(
                tc.tile_pool(name="ps_io", bufs=6, space="PSUM")
            )
            for d in range(6, -1, -1):
                npar = 1 << d
                n = npar * 8
                chn = 2 * n
                off = 8 * ((1 << d) - 1)
                Hch, Cch = H[d + 1], C[d + 1]
                Cev, Cod = Cch[:, :, 0::2, :], Cch[:, :, 1::2, :]

                # iou psum prologue: W_iou emb[par] (independent of H)
                pss = {}
                for g in (0, 2, 1):
                    for m in range(KH):
                        m9 = 3 * g + m
                        ps = ps_io.tile(
                            [CH, npar, 8], F32, tag="ps_io", name="psg"
                        )
                        for k2 in range(KE):
                            nc.tensor.matmul(
                                ps[:],
                                wiou_t[:, k2, CH * m9 : CH * (m9 + 1)],
                                embi_t[:, k2, off : off + n],
                                start=(k2 == 0),
                                stop=False,
                            )
                        pss[m9] = ps

                hsum = tr.tile([CH, KH, npar, 8], BF16, tag="hsum", name="hsum")
                nc.vector.tensor_add(
                    hsum[:], Hch[:, :, 0::2, :], Hch[:, :, 1::2, :]
                )

                # forget gates: psum preloaded with wxf, then U_f matmuls
                # accumulate; f = sigma(psum). Layout [CH, KH, npar, 2, 8].
                f2 = tr.tile([CH, KH, npar, 2, 8], F32, tag="f2", name="f2")
                nchunk = max(n // 256, 1)  # child cols per psum <= 512
                cpn = npar // nchunk
                for m in range(KH):
                    for h in range(nchunk):
                        ps = ps_uf.tile(
                            [CH, cpn, 2, 8], F32, tag="ps_uf", name="psu"
                        )
                        wxfs = wxf_t[
                            :, m, off + 8 * cpn * h : off + 8 * cpn * (h + 1)
                        ].rearrange("p (n e) -> p n e", e=8)
                        nc.scalar.copy(ps[:, :, 0, :], wxfs)
                        nc.scalar.copy(ps[:, :, 1, :], wxfs)
                        for k in range(KH):
                            nc.tensor.matmul(
                                ps[:],
                                uf_t[:, k, CH * m : CH * (m + 1)],
                                Hch[:, k, 2 * cpn * h : 2 * cpn * (h + 1), :],
                                start=False,
                                stop=(k == KH - 1),
                                skip_group_check=True,
                            )
                        nc.scalar.activation(
                            f2[:, m, cpn * h : cpn * (h + 1), :, :],
                            ps[:],
                            AF.Sigmoid,
                        )
                t1 = tr.tile([CH, KH, npar, 8], F32, tag="t1", name="t1")
                t2 = tr.tile([CH, KH, npar, 8], F32, tag="t2", name="t2")
                nc.gpsimd.tensor_mul(t1[:], f2[:, :, :, 0, :], Cev)
                nc.gpsimd.tensor_mul(t2[:], f2[:, :, :, 1, :], Cod)
                fc = tr.tile([CH, KH, npar, 8], F32, tag="fc", name="fc")
                nc.gpsimd.tensor_add(_fl(fc[:]), _fl(t1[:]), _fl(t2[:]))

                # iou gates: psum = W_iou emb[par] + U_iou hsum; gate = ACT
                gates = {}
                for g in (0, 2, 1):
                    gt = tr.tile(
                        [CH, KH, npar, 8], BF16, tag=f"g{g}", name="gt"
                    )
                    for m in range(KH):
                        m9 = 3 * g + m
                        ps = pss[m9]
                        for k in range(KH):
                            nc.tensor.matmul(
                                ps[:],
                                uiou_t[:, k, CH * m9 : CH * (m9 + 1)],
                                hsum[:, k, :, :],
                                start=False,
                                stop=(k == KH - 1),
                            )
                        if g == 2:
                            bias, scale = biou2_t[:, m : m + 1], 2.0
                        else:
                            bias, scale = biou_t[:, m9 : m9 + 1], 1.0
                        nc.scalar.activation(
                            _fl(gt[:, m, :, :]),
                            _fl(ps[:]),
                            AF.Sigmoid,
                            bias=bias,
                            scale=scale,
                        )
                    gates[g] = gt
                gi, go, gu = gates[0], gates[1], gates[2]

                tci = tr.tile([CH, KH, npar, 8], BF16, tag="tci", name="tci")
                nc.vector.tensor_mul(_fl(tci[:]), _fl(gi[:]), _fl(gu[:]))
                ctmp = tr.tile([CH, KH, npar, 8], F32, tag="ctmp", name="ctmp")
                nc.vector.scalar_tensor_tensor(
                    _fl(ctmp[:]), _fl(tci[:]), 2.0, _fl(gi[:]),
                    OP.mult, OP.subtract
                )
                C[d] = acts.tile(
                    [CH, KH, npar, 8], BF16, tag=f"C{d}", name=f"C{d}"
                )
                nc.vector.tensor_add(_fl(C[d][:]), _fl(ctmp[:]), _fl(fc[:]))
                sc = tr.tile([CH, KH, npar, 8], BF16, tag="sc", name="sc")
                nc.scalar.activation(_fl(sc[:]), _fl(C[d][:]), AF.Sigmoid, scale=2.0)
                th = tr.tile([CH, KH, npar, 8], BF16, tag="th", name="th")
                nc.vector.tensor_mul(_fl(th[:]), _fl(go[:]), _fl(sc[:]))
                H[d] = acts.tile(
                    [CH, KH, npar, 8], BF16, tag=f"H{d}", name=f"H{d}"
                )
                nc.vector.scalar_tensor_tensor(
                    _fl(H[d][:]), _fl(th[:]), 2.0, _fl(go[:]),
                    OP.mult, OP.subtract
                )

            # ---- head: logits + log_softmax ----
            ps = ps_io.tile([BL, NCLS], F32, tag="ps_io", name="pshead")
            for k in range(KH):
                nc.tensor.matmul(
                    ps[:],
                    H[0][:, k, 0, :],
                    wout_t[:, k, :],
                    start=(k == 0),
                    stop=False,
                )
            nc.tensor.matmul(
                ps[:], ones_t[:], bout_t[:], start=False, stop=True
            )

            m_t = sm.tile([BL, 1], F32, tag="m")
            nc.vector.tensor_reduce(
                m_t[:], ps[:], mybir.AxisListType.X, OP.max
            )
            negm_t = sm.tile([BL, 1], F32, tag="negm")
            nc.vector.tensor_scalar_mul(negm_t[:], m_t[:], -1.0)
            e_t = sm.tile([BL, NCLS], F32, tag="e")
            nc.scalar.activation(e_t[:], ps[:], AF.Exp, bias=negm_t[:])
            s_t = sm.tile([BL, 1], F32, tag="s")
            nc.vector.tensor_reduce(
                s_t[:], e_t[:], mybir.AxisListType.X, OP.add
            )
            u_t = sm.tile([BL, 1], F32, tag="u")
            nc.vector.tensor_scalar_add(u_t[:], s_t[:], -1.0)
            # ln(1+u) = u * P(u), Horner
            cs = _log_coeffs()  # c0..c4
            acc = sm.tile([BL, 1], F32, tag="acc")
            nc.vector.tensor_scalar(
                acc[:], u_t[:], float(cs[4]), float(cs[3]), OP.mult, OP.add
            )
            for ci in (2, 1, 0):
                tmp = sm.tile([BL, 1], F32, tag=f"tmp{ci}", name="tmp")
                nc.vector.tensor_mul(tmp[:], acc[:], u_t[:])
                acc = sm.tile([BL, 1], F32, tag=f"acc{ci}", name="acc")
                nc.vector.tensor_scalar_add(acc[:], tmp[:], float(cs[ci]))
            logs_t = sm.tile([BL, 1], F32, tag="logs")
            nc.vector.tensor_mul(logs_t[:], acc[:], u_t[:])
            lshift = sm.tile([BL, NCLS], F32, tag="lshift")
            nc.vector.tensor_scalar(
                lshift[:], ps[:], negm_t[:], None, OP.add
            )
            out_t = sm.tile([BL, NCLS], F32, tag="out")
            nc.vector.tensor_scalar(
                out_t[:], lshift[:], logs_t[:], None, OP.subtract
            )
            nc.sync.dma_start(out_d[:], out_t[:])

        # ---- debug taps ----
        if TAPS:
            nc.sync.dma_start(taps["embl"][:], embl_t[:])
            nc.sync.dma_start(taps["embi"][:], embi_t[:])
            nc.sync.dma_start(taps["wxf"][:], wxf_t[:])
            nc.sync.dma_start(taps["H7"][:], H[7][:])
            nc.sync.dma_start(taps["C7"][:], C[7][:])
            nc.sync.dma_start(taps["H5"][:], H[5][:])
            nc.sync.dma_start(taps["H0"][:], H[0][:])

    nc.compile()
    return nc


_CACHE = {}


def _get_nc():
    if "nc" not in _CACHE:
        _CACHE["nc"] = _build()
    return _CACHE["nc"]


def _wrap_idx(tokens):
    """int16 tokens [1024] -> dma_gather idx layout [128, 64]."""
    w = tokens.reshape(64, 16).T  # [16, 64]
    return np.ascontiguousarray(np.tile(w, (8, 1)))  # [128, 64]


def kernel(x, parent, depth, embed, W_iou, U_iou, b_iou, W_f, U_f, b_f,
           W_out, b_out):
    x = np.asarray(x)
    embed = np.asarray(embed, dtype=np.float32)
    embp = np.zeros((V, ROWPAD), np.float32)
    embp[:, :E] = embed
    x16 = x.astype(np.int16)

    biou_h = np.ascontiguousarray(
        np.asarray(b_iou, np.float32).reshape(M9, CH).T
    )
    bf_h = np.ascontiguousarray(np.asarray(b_f, np.float32).reshape(KH, CH).T)
    shared = {
        "embp": embp,
        "wiou": np.asarray(W_iou, np.float32),
        "uiou": np.asarray(U_iou, np.float32),
        "biou": biou_h,
        "wf": np.asarray(W_f, np.float32),
        "uf": np.asarray(U_f, np.float32),
        "bf": bf_h,
        "wout": np.asarray(W_out, np.float32),
        "bout": np.asarray(b_out, np.float32).reshape(1, NCLS),
        "ident": np.eye(128, dtype=np.float32),
    }
    in_maps = []
    for c in range(NCORES):
        xc = x16[:, BL * c : BL * (c + 1)]  # [255, 8]
        leaf = np.ascontiguousarray(xc[127:255]).reshape(-1)  # 1024
        internal = np.concatenate(
            [np.ascontiguousarray(xc[0:127]).reshape(-1), np.zeros(8, np.int16)]
        )
        im = dict(shared)
        im["idxl"] = _wrap_idx(leaf)
        im["idxi"] = _wrap_idx(internal)
        in_maps.append(im)

    nc = _get_nc()
    res = run_bass_kernel_spmd(nc, in_maps, core_ids=list(range(NCORES)))
    kernel._last = res
    out = np.concatenate(
        [np.asarray(res.results[c]["out"]) for c in range(NCORES)], axis=0
    )
    return out.astype(np.float32)


kernel._last = None

